# revision 14
# baseline (speedup 1.0000x reference)
"""Trainium2 Bass kernel for EnhancedCrossAttention3D.

Computes, per batch b:
    q = Wq @ x1 + bq            (x1 = branch1[b] reshaped [C, N])
    k = Wk @ x2 + bk
    v = Wv @ x2 + bv
    attn = softmax((q^T k) / sqrt(C), axis=keys)
    out = Wp @ (attn @ v^T)^T + bp      -> [C, N]

Sharding: 8 cores = 2 batches x 4 query shards of 2048. Each core gets its
full K/V source (branch2[b]) and its query shard; no collectives.

On-core algorithm (flash-style, S^T layout):
    S^T[m, n] = sum_c k[c, m] * qT[c, n]   (m = key index on partitions)
    E = exp(S^T / 8)                       (logits are tiny; no max-sub needed)
    PV[c, n]  = sum_m [v | 1][m, c] * E[m, n]   -> row 64 is the softmax denom
    out[o, n] = (Wp @ PV[0:64]) / denom + (Wp @ bv + bp)
(bv is folded in after normalization: attn rows sum to 1.)
"""

import numpy as np
from contextlib import ExitStack

import concourse.bass as bass
import concourse.mybir as mybir
import concourse.tile as tile
from concourse import bacc
from concourse.bass import ts
from concourse.bass_utils import run_bass_kernel_spmd

B, C, D, H, W = 2, 64, 8, 32, 32
N = D * H * W              # 8192 keys per batch
NCORES = 8
QSH = (B * N) // NCORES    # 2048 queries per core
MCH = N // 128             # 64 key chunks of 128
NT = QSH // 512            # 4 query tiles of 512
F32 = mybir.dt.float32
F32R = mybir.dt.float32r
BF16 = mybir.dt.bfloat16
AF = mybir.ActivationFunctionType

_CACHE = {}


def _emit(tc, xq, xkv, wq, wk, wv, wp, bq, bk, bv, bp, out):
    nc = tc.nc
    ctx = ExitStack()
    # float32r is bit-identical to float32 storage; it only selects the PE's
    # full-rate fp32 streaming mode, so these writes lose no precision.
    ctx.enter_context(nc.allow_low_precision(reason="float32r == float32 bits"))
    const = ctx.enter_context(tc.tile_pool(name="const", bufs=1))
    big = ctx.enter_context(tc.tile_pool(name="big", bufs=1))
    ps = ctx.enter_context(tc.tile_pool(name="ps", bufs=2, space="PSUM"))
    ps_acc_p = ctx.enter_context(tc.tile_pool(name="ps_acc", bufs=1, space="PSUM"))
    ex_pool = ctx.enter_context(tc.tile_pool(name="ex", bufs=3))
    small = ctx.enter_context(tc.tile_pool(name="small", bufs=4))

    # ---- loads ----
    xq_sb = big.tile([C, QSH], F32R)
    nc.sync.dma_start(out=xq_sb, in_=xq)
    xkv_sb = big.tile([C, N], F32R)
    nc.sync.dma_start(out=xkv_sb, in_=xkv)
    wqT = const.tile([C, C], F32R)
    nc.sync.dma_start(out=wqT, in_=wq.rearrange("o c -> c o"))
    wkT = const.tile([C, C], F32R)
    nc.sync.dma_start(out=wkT, in_=wk.rearrange("o c -> c o"))
    wvT = const.tile([C, C], F32R)
    nc.sync.dma_start(out=wvT, in_=wv.rearrange("o c -> c o"))
    wpT = const.tile([C, C], F32R)
    nc.sync.dma_start(out=wpT, in_=wp.rearrange("o c -> c o"))
    bq_sb = const.tile([C, 1], F32)
    nc.sync.dma_start(out=bq_sb, in_=bq.rearrange("(c one) -> c one", one=1))
    bk_sb = const.tile([C, 1], F32)
    nc.sync.dma_start(out=bk_sb, in_=bk.rearrange("(c one) -> c one", one=1))
    bv_sb = const.tile([C, 1], F32)
    nc.sync.dma_start(out=bv_sb, in_=bv.rearrange("(c one) -> c one", one=1))
    bp_sb = const.tile([C, 1], F32)
    nc.sync.dma_start(out=bp_sb, in_=bp.rearrange("(c one) -> c one", one=1))
    # memset can't target f32r (it bitcasts internally); stage ones in f32
    ones_f32 = const.tile([128, MCH], F32)
    nc.vector.memset(ones_f32, 1.0)

    # ---- projections ----
    # qT[o, n] on partitions o
    qT_sb = big.tile([C, QSH], BF16)
    for t in range(NT):
        pq = ps.tile([128, 1024], F32, tag="ps")
        nc.tensor.matmul(pq[0:C, 0:512], lhsT=wqT, rhs=xq_sb[:, ts(t, 512)],
                         start=True, stop=True)
        nc.vector.tensor_scalar_add(qT_sb[:, ts(t, 512)], pq[0:C, 0:512], bq_sb)
    # k[o, m] on partitions o
    k_sb = big.tile([C, N], BF16)
    for t in range(N // 512):
        pk = ps.tile([128, 1024], F32, tag="ps")
        nc.tensor.matmul(pk[0:C, 0:512], lhsT=wkT, rhs=xkv_sb[:, ts(t, 512)],
                         start=True, stop=True)
        nc.vector.tensor_scalar_add(k_sb[:, ts(t, 512)], pk[0:C, 0:512], bk_sb)
    # v[m, c] chunks of 128 keys, with an extra ones column (denominator)
    v_sb = big.tile([128, MCH, C + 1], BF16)
    nc.vector.tensor_copy(v_sb[:, :, C], ones_f32)

    # ---- flash loop over key chunks ----
    ps_acc = ps_acc_p.tile([C + 1, QSH], F32)
    for i in range(MCH):
        # v chunk i: overlaps the S/exp pipeline instead of a serial prologue
        pv = ps.tile([128, 1024], F32, tag="ps")
        nc.tensor.matmul(pv[:, 0:C], lhsT=xkv_sb[:, ts(i, 128)], rhs=wvT,
                         start=True, stop=True)
        nc.vector.tensor_copy(v_sb[:, i, 0:C], pv[:, 0:C])
        for h in range(2):
            s_ps = ps.tile([128, 1024], F32, tag="ps")
            for sub in range(2):
                nt = h * 2 + sub
                nc.tensor.matmul(s_ps[:, ts(sub, 512)],
                                 lhsT=k_sb[:, ts(i, 128)],
                                 rhs=qT_sb[:, ts(nt, 512)],
                                 start=True, stop=True)
            ex = ex_pool.tile([128, 1024], BF16)
            nc.scalar.activation(ex, s_ps, AF.Exp, scale=0.125)
            for sub in range(2):
                nt = h * 2 + sub
                nc.tensor.matmul(ps_acc[:, ts(nt, 512)], lhsT=v_sb[:, i, :],
                                 rhs=ex[:, ts(sub, 512)],
                                 start=(i == 0), stop=(i == MCH - 1),
                                 skip_group_check=True)

    # ---- epilogue: normalize (+bv), project, +bp ----
    denom = small.tile([1, QSH], F32, tag="denom")
    nc.vector.tensor_copy(denom, ps_acc[C:C + 1, :])
    rbd = small.tile([C, QSH], F32, tag="rbd")
    nc.gpsimd.partition_broadcast(rbd, denom)
    rb = small.tile([C, QSH], F32, tag="rb")
    nc.vector.reciprocal(rb, rbd)
    for t in range(NT):
        pvn = small.tile([C, 512], F32R, tag="pvn")
        nc.vector.tensor_mul(pvn, ps_acc[0:C, ts(t, 512)], rb[:, ts(t, 512)])
        nc.vector.tensor_scalar_add(pvn, pvn, bv_sb)
        pp = ps.tile([128, 1024], F32, tag="ps")
        nc.tensor.matmul(pp[0:C, 0:512], lhsT=wpT, rhs=pvn,
                         start=True, stop=True)
        o_sb = small.tile([C, 512], F32, tag="o")
        nc.scalar.activation(o_sb, pp[0:C, 0:512], AF.Identity, bias=bp_sb,
                             scale=1.0)
        nc.sync.dma_start(out=out[:, ts(t, 512)], in_=o_sb)
    ctx.close()


def _build():
    nc = bacc.Bacc("TRN2", target_bir_lowering=False, debug=False,
                   num_devices=NCORES)
    aps = {}
    aps["xq"] = nc.dram_tensor("xq", [C, QSH], F32R, kind="ExternalInput").ap()
    aps["xkv"] = nc.dram_tensor("xkv", [C, N], F32R, kind="ExternalInput").ap()
    for nm in ("wq", "wk", "wv", "wp"):
        aps[nm] = nc.dram_tensor(nm, [C, C], F32R, kind="ExternalInput").ap()
    for nm in ("bq", "bk", "bv", "bp"):
        aps[nm] = nc.dram_tensor(nm, [C], F32, kind="ExternalInput").ap()
    aps["out"] = nc.dram_tensor("out", [C, QSH], F32, kind="ExternalOutput").ap()
    with tile.TileContext(nc) as tc:
        _emit(tc, **aps)
    nc.finalize()
    return nc


def kernel(branch1, branch2, Wq, bq, Wk, bk, Wv, bv, Wp, bp, **run_kwargs):
    if "nc" not in _CACHE:
        _CACHE["nc"] = _build()
    nc = _CACHE["nc"]

    x1 = np.ascontiguousarray(np.asarray(branch1, np.float32).reshape(B, C, N))
    x2 = np.ascontiguousarray(np.asarray(branch2, np.float32).reshape(B, C, N))
    consts = {
        "wq": np.ascontiguousarray(Wq, np.float32),
        "wk": np.ascontiguousarray(Wk, np.float32),
        "wv": np.ascontiguousarray(Wv, np.float32),
        "wp": np.ascontiguousarray(Wp, np.float32),
        "bq": np.ascontiguousarray(bq, np.float32),
        "bk": np.ascontiguousarray(bk, np.float32),
        "bv": np.ascontiguousarray(bv, np.float32),
        "bp": np.ascontiguousarray(bp, np.float32),
    }
    in_maps = []
    for core in range(NCORES):
        b, s = divmod(core, NCORES // B)
        in_maps.append({
            "xq": np.ascontiguousarray(x1[b, :, s * QSH:(s + 1) * QSH]),
            "xkv": x2[b],
            **consts,
        })
    res = run_bass_kernel_spmd(nc, in_maps, core_ids=list(range(NCORES)),
                               **run_kwargs)
    out = np.empty((B, C, N), np.float32)
    for core in range(NCORES):
        b, s = divmod(core, NCORES // B)
        out[b, :, s * QSH:(s + 1) * QSH] = res.results[core]["out"]
    if run_kwargs:
        _CACHE["last_result"] = res
    return out.reshape(B, C, D, H, W)


# revision 15
# speedup vs baseline: 1.1440x; 1.1440x over previous
"""Trainium2 Bass kernel for EnhancedCrossAttention3D.

Computes, per batch b:
    q = Wq @ x1 + bq            (x1 = branch1[b] reshaped [C, N])
    k = Wk @ x2 + bk
    v = Wv @ x2 + bv
    attn = softmax((q^T k) / sqrt(C), axis=keys)
    out = Wp @ (attn @ v^T)^T + bp      -> [C, N]

Sharding: 8 cores = 2 batches x 4 query shards of 2048. Each core gets its
full K/V source (branch2[b]) and its query shard; no collectives.

On-core algorithm (flash-style, S^T layout):
    S^T[m, n] = sum_c k[c, m] * qT[c, n]   (m = key index on partitions)
    E = exp(S^T / 8)                       (logits are tiny; no max-sub needed)
    PV[c, n]  = sum_m [v | 1][m, c] * E[m, n]   -> row 64 is the softmax denom
    out[o, n] = (Wp @ PV[0:64]) / denom + (Wp @ bv + bp)
(bv is folded in after normalization: attn rows sum to 1.)
"""

import numpy as np
from contextlib import ExitStack

import concourse.bass as bass
import concourse.mybir as mybir
import concourse.tile as tile
from concourse import bacc
from concourse.bass import ts
from concourse.bass_utils import run_bass_kernel_spmd

B, C, D, H, W = 2, 64, 8, 32, 32
N = D * H * W              # 8192 keys per batch
NCORES = 8
QSH = (B * N) // NCORES    # 2048 queries per core
MCH = N // 128             # 64 key chunks of 128
NT = QSH // 512            # 4 query tiles of 512
F32 = mybir.dt.float32
F32R = mybir.dt.float32r
BF16 = mybir.dt.bfloat16
AF = mybir.ActivationFunctionType

_CACHE = {}


def _emit(tc, xq, xkv, wq, wk, wv, wp, bq, bk, bv, bp, out):
    nc = tc.nc
    ctx = ExitStack()
    # float32r is bit-identical to float32 storage; it only selects the PE's
    # full-rate fp32 streaming mode, so these writes lose no precision.
    ctx.enter_context(nc.allow_low_precision(reason="float32r == float32 bits"))
    const = ctx.enter_context(tc.tile_pool(name="const", bufs=1))
    big = ctx.enter_context(tc.tile_pool(name="big", bufs=1))
    ps = ctx.enter_context(tc.tile_pool(name="ps", bufs=2, space="PSUM"))
    ps_acc_p = ctx.enter_context(tc.tile_pool(name="ps_acc", bufs=1, space="PSUM"))
    ex_pool = ctx.enter_context(tc.tile_pool(name="ex", bufs=3))
    small = ctx.enter_context(tc.tile_pool(name="small", bufs=4))

    # ---- loads ----
    xq_sb = big.tile([C, QSH], F32R)
    nc.sync.dma_start(out=xq_sb, in_=xq)
    xkv_sb = big.tile([C, N], F32R)
    nc.sync.dma_start(out=xkv_sb, in_=xkv)
    wqT = const.tile([C, C], F32R)
    nc.sync.dma_start(out=wqT, in_=wq.rearrange("o c -> c o"))
    wkT = const.tile([C, C], F32R)
    nc.sync.dma_start(out=wkT, in_=wk.rearrange("o c -> c o"))
    wvT = const.tile([C, C], F32R)
    nc.sync.dma_start(out=wvT, in_=wv.rearrange("o c -> c o"))
    wpT = const.tile([C, C], F32R)
    nc.sync.dma_start(out=wpT, in_=wp.rearrange("o c -> c o"))
    bq_sb = const.tile([C, 1], F32)
    nc.sync.dma_start(out=bq_sb, in_=bq.rearrange("(c one) -> c one", one=1))
    bk_sb = const.tile([C, 1], F32)
    nc.sync.dma_start(out=bk_sb, in_=bk.rearrange("(c one) -> c one", one=1))
    bv_sb = const.tile([C, 1], F32)
    nc.sync.dma_start(out=bv_sb, in_=bv.rearrange("(c one) -> c one", one=1))
    bp_sb = const.tile([C, 1], F32)
    nc.sync.dma_start(out=bp_sb, in_=bp.rearrange("(c one) -> c one", one=1))
    # memset can't target f32r (it bitcasts internally); stage ones in f32
    ones_f32 = const.tile([128, MCH], F32)
    nc.vector.memset(ones_f32, 1.0)

    # ---- projections ----
    # qT[o, n] on partitions o
    qT_sb = big.tile([C, QSH], BF16)
    for t in range(NT):
        pq = ps.tile([128, 1024], F32, tag="ps")
        nc.tensor.matmul(pq[0:C, 0:512], lhsT=wqT, rhs=xq_sb[:, ts(t, 512)],
                         start=True, stop=True)
        nc.vector.tensor_scalar_add(qT_sb[:, ts(t, 512)], pq[0:C, 0:512], bq_sb)
    # k[o, m] on partitions o
    k_sb = big.tile([C, N], BF16)
    for t in range(N // 512):
        pk = ps.tile([128, 1024], F32, tag="ps")
        nc.tensor.matmul(pk[0:C, 0:512], lhsT=wkT, rhs=xkv_sb[:, ts(t, 512)],
                         start=True, stop=True)
        nc.vector.tensor_scalar_add(k_sb[:, ts(t, 512)], pk[0:C, 0:512], bk_sb)
    # v[m, c] chunks of 128 keys, with an extra ones column (denominator)
    v_sb = big.tile([128, MCH, C + 1], BF16)
    nc.vector.tensor_copy(v_sb[:, :, C], ones_f32)

    # ---- flash loop over key chunks ----
    ps_acc = ps_acc_p.tile([C + 1, QSH], F32)
    for i in range(MCH):
        if i % 4 == 0:
            # v chunks i..i+3 batched into one psum slot: 4 matmuls, 1 cast.
            # Interleaved with the S/exp pipeline instead of a serial prologue.
            g = i // 4
            pv = ps.tile([128, 1024], F32, tag="ps")
            for j in range(4):
                nc.tensor.matmul(pv[:, ts(j, C)],
                                 lhsT=xkv_sb[:, ts(4 * g + j, 128)], rhs=wvT,
                                 start=True, stop=True)
            nc.vector.tensor_copy(
                v_sb[:, 4 * g:4 * g + 4, 0:C],
                pv[:, 0:4 * C].rearrange("p (g c) -> p g c", c=C))
        for h in range(2):
            s_ps = ps.tile([128, 1024], F32, tag="ps")
            for sub in range(2):
                nt = h * 2 + sub
                nc.tensor.matmul(s_ps[:, ts(sub, 512)],
                                 lhsT=k_sb[:, ts(i, 128)],
                                 rhs=qT_sb[:, ts(nt, 512)],
                                 start=True, stop=True)
            ex = ex_pool.tile([128, 1024], BF16)
            nc.scalar.activation(ex, s_ps, AF.Exp, scale=0.125)
            for sub in range(2):
                nt = h * 2 + sub
                nc.tensor.matmul(ps_acc[:, ts(nt, 512)], lhsT=v_sb[:, i, :],
                                 rhs=ex[:, ts(sub, 512)],
                                 start=(i == 0), stop=(i == MCH - 1),
                                 skip_group_check=True)

    # ---- epilogue: normalize (+bv), project, +bp ----
    denom = small.tile([1, QSH], F32, tag="denom")
    nc.vector.tensor_copy(denom, ps_acc[C:C + 1, :])
    rbd = small.tile([C, QSH], F32, tag="rbd")
    nc.gpsimd.partition_broadcast(rbd, denom)
    rb = small.tile([C, QSH], F32, tag="rb")
    nc.vector.reciprocal(rb, rbd)
    for t in range(NT):
        pvn = small.tile([C, 512], F32R, tag="pvn")
        nc.vector.tensor_mul(pvn, ps_acc[0:C, ts(t, 512)], rb[:, ts(t, 512)])
        nc.vector.tensor_scalar_add(pvn, pvn, bv_sb)
        pp = ps.tile([128, 1024], F32, tag="ps")
        nc.tensor.matmul(pp[0:C, 0:512], lhsT=wpT, rhs=pvn,
                         start=True, stop=True)
        o_sb = small.tile([C, 512], F32, tag="o")
        nc.scalar.activation(o_sb, pp[0:C, 0:512], AF.Identity, bias=bp_sb,
                             scale=1.0)
        nc.sync.dma_start(out=out[:, ts(t, 512)], in_=o_sb)
    ctx.close()


def _build():
    nc = bacc.Bacc("TRN2", target_bir_lowering=False, debug=False,
                   num_devices=NCORES)
    aps = {}
    aps["xq"] = nc.dram_tensor("xq", [C, QSH], F32R, kind="ExternalInput").ap()
    aps["xkv"] = nc.dram_tensor("xkv", [C, N], F32R, kind="ExternalInput").ap()
    for nm in ("wq", "wk", "wv", "wp"):
        aps[nm] = nc.dram_tensor(nm, [C, C], F32R, kind="ExternalInput").ap()
    for nm in ("bq", "bk", "bv", "bp"):
        aps[nm] = nc.dram_tensor(nm, [C], F32, kind="ExternalInput").ap()
    aps["out"] = nc.dram_tensor("out", [C, QSH], F32, kind="ExternalOutput").ap()
    with tile.TileContext(nc) as tc:
        _emit(tc, **aps)
    nc.finalize()
    return nc


def kernel(branch1, branch2, Wq, bq, Wk, bk, Wv, bv, Wp, bp, **run_kwargs):
    if "nc" not in _CACHE:
        _CACHE["nc"] = _build()
    nc = _CACHE["nc"]

    x1 = np.ascontiguousarray(np.asarray(branch1, np.float32).reshape(B, C, N))
    x2 = np.ascontiguousarray(np.asarray(branch2, np.float32).reshape(B, C, N))
    consts = {
        "wq": np.ascontiguousarray(Wq, np.float32),
        "wk": np.ascontiguousarray(Wk, np.float32),
        "wv": np.ascontiguousarray(Wv, np.float32),
        "wp": np.ascontiguousarray(Wp, np.float32),
        "bq": np.ascontiguousarray(bq, np.float32),
        "bk": np.ascontiguousarray(bk, np.float32),
        "bv": np.ascontiguousarray(bv, np.float32),
        "bp": np.ascontiguousarray(bp, np.float32),
    }
    in_maps = []
    for core in range(NCORES):
        b, s = divmod(core, NCORES // B)
        in_maps.append({
            "xq": np.ascontiguousarray(x1[b, :, s * QSH:(s + 1) * QSH]),
            "xkv": x2[b],
            **consts,
        })
    res = run_bass_kernel_spmd(nc, in_maps, core_ids=list(range(NCORES)),
                               **run_kwargs)
    out = np.empty((B, C, N), np.float32)
    for core in range(NCORES):
        b, s = divmod(core, NCORES // B)
        out[b, :, s * QSH:(s + 1) * QSH] = res.results[core]["out"]
    if run_kwargs:
        _CACHE["last_result"] = res
    return out.reshape(B, C, D, H, W)


# revision 16
# speedup vs baseline: 1.2341x; 1.0788x over previous
"""Trainium2 Bass kernel for EnhancedCrossAttention3D.

Computes, per batch b:
    q = Wq @ x1 + bq            (x1 = branch1[b] reshaped [C, N])
    k = Wk @ x2 + bk
    v = Wv @ x2 + bv
    attn = softmax((q^T k) / sqrt(C), axis=keys)
    out = Wp @ (attn @ v^T)^T + bp      -> [C, N]

Sharding: 8 cores = 2 batches x 4 query shards of 2048. Each core gets its
full K/V source (branch2[b]) and its query shard; no collectives.

On-core algorithm (flash-style, S^T layout):
    S^T[m, n] = sum_c k[c, m] * qT[c, n]   (m = key index on partitions)
    E = exp(S^T / 8)                       (logits are tiny; no max-sub needed)
    PV[c, n]  = sum_m [v | 1][m, c] * E[m, n]   -> row 64 is the softmax denom
    out[o, n] = Wp @ (PV[0:64] / denom + bv) + bp
(bv is folded in after normalization: attn rows sum to 1.)

Structure: queries are processed in two 1024-wide half-passes so the PV
accumulator needs only 2 PSUM banks, freeing 6 banks for a triple-buffered
S^T pipeline (keeps the PE continuously fed -> HAM stays at full clock).
Matmul operands are bf16 (full-rate streaming + fast weight load); exp runs
on the Scalar engine straight out of PSUM; PSUM accumulation is fp32.
"""

import numpy as np
from contextlib import ExitStack

import concourse.bass as bass
import concourse.mybir as mybir
import concourse.tile as tile
from concourse import bacc
from concourse.bass import ts
from concourse.bass_utils import run_bass_kernel_spmd

B, C, D, H, W = 2, 64, 8, 32, 32
N = D * H * W              # 8192 keys per batch
NCORES = 8
QSH = (B * N) // NCORES    # 2048 queries per core
MCH = N // 128             # 64 key chunks of 128
NH = QSH // 1024           # 2 query half-passes
F32 = mybir.dt.float32
F32R = mybir.dt.float32r
BF16 = mybir.dt.bfloat16
AF = mybir.ActivationFunctionType

_CACHE = {}


def _emit(tc, xq, xkv, wq, wk, wv, wp, bq, bk, bv, bp, out):
    nc = tc.nc
    ctx = ExitStack()
    # bf16/f32r writes below intentionally round fp32; errors wash out in the
    # 8192-term attention sums and sit ~1e-4 of output scale.
    ctx.enter_context(nc.allow_low_precision(reason="bf16 attention operands"))
    const = ctx.enter_context(tc.tile_pool(name="const", bufs=1))
    big = ctx.enter_context(tc.tile_pool(name="big", bufs=1))
    ps3 = ctx.enter_context(tc.tile_pool(name="ps3", bufs=3, space="PSUM"))
    acc_p = ctx.enter_context(tc.tile_pool(name="acc", bufs=1, space="PSUM"))
    ex_pool = ctx.enter_context(tc.tile_pool(name="ex", bufs=4))
    small = ctx.enter_context(tc.tile_pool(name="small", bufs=4))

    # ---- loads (gpsimd DMAs cast f32 -> bf16 in flight) ----
    xq_bf = big.tile([C, QSH], BF16)
    for s in range(2):
        nc.gpsimd.dma_start(out=xq_bf[:, ts(s, QSH // 2)],
                            in_=xq[:, ts(s, QSH // 2)])
    xkv_bf = big.tile([C, N], BF16)
    for s in range(4):
        nc.gpsimd.dma_start(out=xkv_bf[:, ts(s, N // 4)],
                            in_=xkv[:, ts(s, N // 4)])
    wqT = const.tile([C, C], BF16)
    nc.gpsimd.dma_start(out=wqT, in_=wq.rearrange("o c -> c o"))
    wkT = const.tile([C, C], BF16)
    nc.gpsimd.dma_start(out=wkT, in_=wk.rearrange("o c -> c o"))
    wvT = const.tile([C, C], BF16)
    nc.gpsimd.dma_start(out=wvT, in_=wv.rearrange("o c -> c o"))
    wpT = const.tile([C, C], F32R)
    nc.sync.dma_start(out=wpT, in_=wp.rearrange("o c -> c o"))
    bq_sb = const.tile([C, 1], F32)
    nc.sync.dma_start(out=bq_sb, in_=bq.rearrange("(c one) -> c one", one=1))
    bk_sb = const.tile([C, 1], F32)
    nc.sync.dma_start(out=bk_sb, in_=bk.rearrange("(c one) -> c one", one=1))
    bv_sb = const.tile([C, 1], F32)
    nc.sync.dma_start(out=bv_sb, in_=bv.rearrange("(c one) -> c one", one=1))
    bp_sb = const.tile([C, 1], F32)
    nc.sync.dma_start(out=bp_sb, in_=bp.rearrange("(c one) -> c one", one=1))
    ones_f32 = const.tile([128, MCH], F32)
    nc.vector.memset(ones_f32, 1.0)

    # ---- q/k projections (bf16 out via the DVE bias-add) ----
    qT_sb = big.tile([C, QSH], BF16)
    for t in range(QSH // 512):
        pq = ps3.tile([128, 1024], F32, tag="ps")
        nc.tensor.matmul(pq[0:C, 0:512], lhsT=wqT, rhs=xq_bf[:, ts(t, 512)],
                         start=True, stop=True)
        nc.vector.tensor_scalar_add(qT_sb[:, ts(t, 512)], pq[0:C, 0:512], bq_sb)
    k_sb = big.tile([C, N], BF16)
    for t in range(N // 512):
        pk = ps3.tile([128, 1024], F32, tag="ps")
        nc.tensor.matmul(pk[0:C, 0:512], lhsT=wkT, rhs=xkv_bf[:, ts(t, 512)],
                         start=True, stop=True)
        nc.vector.tensor_scalar_add(k_sb[:, ts(t, 512)], pk[0:C, 0:512], bk_sb)
    # v[m, c] with a ones column (row 64 of PV becomes the softmax denominator)
    v_sb = big.tile([128, MCH, C + 1], BF16)
    nc.vector.tensor_copy(v_sb[:, :, C], ones_f32)

    # ---- flash loop: two query half-passes over all key chunks ----
    for p in range(NH):
        acc = acc_p.tile([C + 1, 1024], F32, tag="acc")
        for i in range(MCH):
            if p == 0 and i % 4 == 0:
                # v chunks i..i+3: 4 matmuls into one psum slot, one cast out
                g = i // 4
                pv = ps3.tile([128, 1024], F32, tag="ps")
                for j in range(4):
                    nc.tensor.matmul(pv[:, ts(j, C)],
                                     lhsT=xkv_bf[:, ts(4 * g + j, 128)],
                                     rhs=wvT, start=True, stop=True)
                nc.vector.tensor_copy(
                    v_sb[:, 4 * g:4 * g + 4, 0:C],
                    pv[:, 0:4 * C].rearrange("p (g c) -> p g c", c=C))
            s_ps = ps3.tile([128, 1024], F32, tag="ps")
            for sub in range(2):
                nc.tensor.matmul(s_ps[:, ts(sub, 512)],
                                 lhsT=k_sb[:, ts(i, 128)],
                                 rhs=qT_sb[:, p * 1024 + 512 * sub:
                                           p * 1024 + 512 * (sub + 1)],
                                 start=True, stop=True)
            ex = ex_pool.tile([128, 1024], BF16)
            nc.scalar.activation(ex, s_ps, AF.Exp, scale=0.125)
            for sub in range(2):
                nc.tensor.matmul(acc[:, ts(sub, 512)], lhsT=v_sb[:, i, :],
                                 rhs=ex[:, ts(sub, 512)],
                                 start=(i == 0), stop=(i == MCH - 1),
                                 skip_group_check=True)

        # ---- per-pass epilogue: normalize (+bv), project, +bp ----
        denom = small.tile([1, 1024], F32, tag="denom")
        nc.vector.tensor_copy(denom, acc[C:C + 1, :])
        rbd = small.tile([C, 1024], F32, tag="rbd")
        nc.gpsimd.partition_broadcast(rbd, denom)
        rb = small.tile([C, 1024], F32, tag="rb")
        nc.vector.reciprocal(rb, rbd)
        for t in range(2):
            nt = p * 2 + t
            pvn = small.tile([C, 512], F32R, tag="pvn")
            nc.vector.tensor_mul(pvn, acc[0:C, ts(t, 512)], rb[:, ts(t, 512)])
            nc.vector.tensor_scalar_add(pvn, pvn, bv_sb)
            pp = ps3.tile([128, 1024], F32, tag="ps")
            nc.tensor.matmul(pp[0:C, 0:512], lhsT=wpT, rhs=pvn,
                             start=True, stop=True)
            o_sb = small.tile([C, 512], F32, tag="o")
            nc.scalar.activation(o_sb, pp[0:C, 0:512], AF.Identity,
                                 bias=bp_sb, scale=1.0)
            nc.sync.dma_start(out=out[:, ts(nt, 512)], in_=o_sb)
    ctx.close()


def _build():
    nc = bacc.Bacc("TRN2", target_bir_lowering=False, debug=False,
                   num_devices=NCORES)
    aps = {}
    aps["xq"] = nc.dram_tensor("xq", [C, QSH], F32, kind="ExternalInput").ap()
    aps["xkv"] = nc.dram_tensor("xkv", [C, N], F32, kind="ExternalInput").ap()
    for nm in ("wq", "wk", "wv"):
        aps[nm] = nc.dram_tensor(nm, [C, C], F32, kind="ExternalInput").ap()
    aps["wp"] = nc.dram_tensor("wp", [C, C], F32R, kind="ExternalInput").ap()
    for nm in ("bq", "bk", "bv", "bp"):
        aps[nm] = nc.dram_tensor(nm, [C], F32, kind="ExternalInput").ap()
    aps["out"] = nc.dram_tensor("out", [C, QSH], F32, kind="ExternalOutput").ap()
    with tile.TileContext(nc) as tc:
        _emit(tc, **aps)
    nc.finalize()
    return nc


def kernel(branch1, branch2, Wq, bq, Wk, bk, Wv, bv, Wp, bp, **run_kwargs):
    if "nc" not in _CACHE:
        _CACHE["nc"] = _build()
    nc = _CACHE["nc"]

    x1 = np.ascontiguousarray(np.asarray(branch1, np.float32).reshape(B, C, N))
    x2 = np.ascontiguousarray(np.asarray(branch2, np.float32).reshape(B, C, N))
    consts = {
        "wq": np.ascontiguousarray(Wq, np.float32),
        "wk": np.ascontiguousarray(Wk, np.float32),
        "wv": np.ascontiguousarray(Wv, np.float32),
        "wp": np.ascontiguousarray(Wp, np.float32),
        "bq": np.ascontiguousarray(bq, np.float32),
        "bk": np.ascontiguousarray(bk, np.float32),
        "bv": np.ascontiguousarray(bv, np.float32),
        "bp": np.ascontiguousarray(bp, np.float32),
    }
    in_maps = []
    for core in range(NCORES):
        b, s = divmod(core, NCORES // B)
        in_maps.append({
            "xq": np.ascontiguousarray(x1[b, :, s * QSH:(s + 1) * QSH]),
            "xkv": x2[b],
            **consts,
        })
    res = run_bass_kernel_spmd(nc, in_maps, core_ids=list(range(NCORES)),
                               **run_kwargs)
    out = np.empty((B, C, N), np.float32)
    for core in range(NCORES):
        b, s = divmod(core, NCORES // B)
        out[b, :, s * QSH:(s + 1) * QSH] = res.results[core]["out"]
    if run_kwargs:
        _CACHE["last_result"] = res
    return out.reshape(B, C, D, H, W)


# revision 17
# speedup vs baseline: 1.7826x; 1.4445x over previous
"""Trainium2 Bass kernel for EnhancedCrossAttention3D.

Computes, per batch b:
    q = Wq @ x1 + bq            (x1 = branch1[b] reshaped [C, N])
    k = Wk @ x2 + bk
    v = Wv @ x2 + bv
    attn = softmax((q^T k) / sqrt(C), axis=keys)
    out = Wp @ (attn @ v^T)^T + bp      -> [C, N]

Sharding: 8 cores = 2 batches x 4 query shards of 2048. Each core gets its
full K/V source (branch2[b]) and its query shard; no collectives.

On-core algorithm (flash-style, S^T layout):
    S^T[m, n] = sum_c k[c, m] * qT[c, n]   (m = key index on partitions)
    E = exp(S^T / 8)                       (logits are tiny; no max-sub needed)
    PV[c, n]  = sum_m [v | 1][m, c] * E[m, n]   -> row 64 is the softmax denom
    out[o, n] = Wp @ (PV[0:64] / denom + bv) + bp
(bv is folded in after normalization: attn rows sum to 1.)

Structure: queries are processed in two 1024-wide half-passes so the PV
accumulator needs only 2 PSUM banks, freeing 6 banks for a triple-buffered
S^T pipeline (keeps the PE continuously fed -> HAM stays at full clock).
Matmul operands are bf16 (full-rate streaming + fast weight load); exp runs
on the Scalar engine straight out of PSUM; PSUM accumulation is fp32.
"""

import numpy as np
from contextlib import ExitStack

import concourse.bass as bass
import concourse.mybir as mybir
import concourse.tile as tile
from concourse import bacc
from concourse.bass import ts
from concourse.bass_utils import run_bass_kernel_spmd

B, C, D, H, W = 2, 64, 8, 32, 32
N = D * H * W              # 8192 keys per batch
NCORES = 8
QSH = (B * N) // NCORES    # 2048 queries per core
MCH = N // 128             # 64 key chunks of 128
NH = QSH // 1024           # 2 query half-passes
F32 = mybir.dt.float32
F32R = mybir.dt.float32r
BF16 = mybir.dt.bfloat16
AF = mybir.ActivationFunctionType

_CACHE = {}


def _emit(tc, xq, xkv, wq, wk, wv, wp, bq, bk, bv, bp, out):
    nc = tc.nc
    ctx = ExitStack()
    # bf16/f32r writes below intentionally round fp32; errors wash out in the
    # 8192-term attention sums and sit ~1e-4 of output scale.
    ctx.enter_context(nc.allow_low_precision(reason="bf16 attention operands"))
    const = ctx.enter_context(tc.tile_pool(name="const", bufs=1))
    big = ctx.enter_context(tc.tile_pool(name="big", bufs=1))
    ps3 = ctx.enter_context(tc.tile_pool(name="ps3", bufs=3, space="PSUM"))
    acc_p = ctx.enter_context(tc.tile_pool(name="acc", bufs=1, space="PSUM"))
    ex_pool = ctx.enter_context(tc.tile_pool(name="ex", bufs=4))
    small = ctx.enter_context(tc.tile_pool(name="small", bufs=4))

    # ---- loads (gpsimd DMAs cast f32 -> bf16 in flight) ----
    xq_bf = big.tile([C, QSH], BF16)
    for s in range(2):
        nc.gpsimd.dma_start(out=xq_bf[:, ts(s, QSH // 2)],
                            in_=xq[:, ts(s, QSH // 2)])
    xkv_bf = big.tile([C, N], BF16)
    for s in range(4):
        nc.gpsimd.dma_start(out=xkv_bf[:, ts(s, N // 4)],
                            in_=xkv[:, ts(s, N // 4)])
    wqT = const.tile([C, C], BF16)
    nc.gpsimd.dma_start(out=wqT, in_=wq.rearrange("o c -> c o"))
    wkT = const.tile([C, C], BF16)
    nc.gpsimd.dma_start(out=wkT, in_=wk.rearrange("o c -> c o"))
    wvT = const.tile([C, C], BF16)
    nc.gpsimd.dma_start(out=wvT, in_=wv.rearrange("o c -> c o"))
    wpT = const.tile([C, C], F32R)
    nc.sync.dma_start(out=wpT, in_=wp.rearrange("o c -> c o"))
    bq_sb = const.tile([C, 1], F32)
    nc.sync.dma_start(out=bq_sb, in_=bq.rearrange("(c one) -> c one", one=1))
    bk_sb = const.tile([C, 1], F32)
    nc.sync.dma_start(out=bk_sb, in_=bk.rearrange("(c one) -> c one", one=1))
    bv_sb = const.tile([C, 1], F32)
    nc.sync.dma_start(out=bv_sb, in_=bv.rearrange("(c one) -> c one", one=1))
    bp_sb = const.tile([C, 1], F32)
    nc.sync.dma_start(out=bp_sb, in_=bp.rearrange("(c one) -> c one", one=1))
    ones_f32 = const.tile([128, MCH], F32)
    nc.vector.memset(ones_f32, 1.0)

    # ---- q/k projections (bf16 out via the DVE bias-add) ----
    # Both live twice (partitions 0-63 and 64-127) so the S^T matmuls can be
    # row-packed: two concurrent K=64 matmuls in opposite PE-array halves.
    qT_sb = big.tile([128, QSH], BF16)
    for t in range(QSH // 512):
        pq = ps3.tile([128, 1024], F32, tag="ps")
        nc.tensor.matmul(pq[0:C, 0:512], lhsT=wqT, rhs=xq_bf[:, ts(t, 512)],
                         start=True, stop=True)
        nc.vector.tensor_scalar_add(qT_sb[0:C, ts(t, 512)], pq[0:C, 0:512],
                                    bq_sb)
        nc.vector.tensor_scalar_add(qT_sb[C:2 * C, ts(t, 512)], pq[0:C, 0:512],
                                    bq_sb)
    k_sb = big.tile([128, N], BF16)
    for t in range(N // 512):
        pk = ps3.tile([128, 1024], F32, tag="ps")
        nc.tensor.matmul(pk[0:C, 0:512], lhsT=wkT, rhs=xkv_bf[:, ts(t, 512)],
                         start=True, stop=True)
        nc.vector.tensor_scalar_add(k_sb[0:C, ts(t, 512)], pk[0:C, 0:512],
                                    bk_sb)
        nc.vector.tensor_scalar_add(k_sb[C:2 * C, ts(t, 512)], pk[0:C, 0:512],
                                    bk_sb)
    # v[m, c] with a ones column (row 64 of PV becomes the softmax denominator)
    v_sb = big.tile([128, MCH, C + 1], BF16)
    nc.vector.tensor_copy(v_sb[:, :, C], ones_f32)

    # ---- flash loop: two query half-passes over all key chunks ----
    for p in range(NH):
        acc = acc_p.tile([C + 1, 1024], F32, tag="acc")
        for i in range(MCH):
            if p == 0 and i % 4 == 0:
                # v chunks i..i+3: 4 matmuls into one psum slot, one cast out
                g = i // 4
                pv = ps3.tile([128, 1024], F32, tag="ps")
                for j in range(4):
                    nc.tensor.matmul(pv[:, ts(j, C)],
                                     lhsT=xkv_bf[:, ts(4 * g + j, 128)],
                                     rhs=wvT, start=True, stop=True)
                nc.vector.tensor_copy(
                    v_sb[:, 4 * g:4 * g + 4, 0:C],
                    pv[:, 0:4 * C].rearrange("p (g c) -> p g c", c=C))
            s_ps = ps3.tile([128, 1024], F32, tag="ps")
            for sub in range(2):
                lo = C * sub
                nc.tensor.matmul(s_ps[:, ts(sub, 512)],
                                 lhsT=k_sb[lo:lo + C, ts(i, 128)],
                                 rhs=qT_sb[lo:lo + C,
                                           p * 1024 + 512 * sub:
                                           p * 1024 + 512 * (sub + 1)],
                                 start=True, stop=True)
            ex = ex_pool.tile([128, 1024], BF16)
            nc.scalar.activation(ex, s_ps, AF.Exp, scale=0.125)
            for sub in range(2):
                nc.tensor.matmul(acc[:, ts(sub, 512)], lhsT=v_sb[:, i, :],
                                 rhs=ex[:, ts(sub, 512)],
                                 start=(i == 0), stop=(i == MCH - 1),
                                 skip_group_check=True)

        # ---- per-pass epilogue: normalize (+bv), project, +bp ----
        denom = small.tile([1, 1024], F32, tag="denom")
        nc.vector.tensor_copy(denom, acc[C:C + 1, :])
        rbd = small.tile([C, 1024], F32, tag="rbd")
        nc.gpsimd.partition_broadcast(rbd, denom)
        rb = small.tile([C, 1024], F32, tag="rb")
        nc.vector.reciprocal(rb, rbd)
        for t in range(2):
            nt = p * 2 + t
            pvn = small.tile([C, 512], F32R, tag="pvn")
            nc.vector.tensor_mul(pvn, acc[0:C, ts(t, 512)], rb[:, ts(t, 512)])
            nc.vector.tensor_scalar_add(pvn, pvn, bv_sb)
            pp = ps3.tile([128, 1024], F32, tag="ps")
            nc.tensor.matmul(pp[0:C, 0:512], lhsT=wpT, rhs=pvn,
                             start=True, stop=True)
            o_sb = small.tile([C, 512], F32, tag="o")
            nc.scalar.activation(o_sb, pp[0:C, 0:512], AF.Identity,
                                 bias=bp_sb, scale=1.0)
            nc.sync.dma_start(out=out[:, ts(nt, 512)], in_=o_sb)
    ctx.close()


def _build():
    nc = bacc.Bacc("TRN2", target_bir_lowering=False, debug=False,
                   num_devices=NCORES)
    aps = {}
    aps["xq"] = nc.dram_tensor("xq", [C, QSH], F32, kind="ExternalInput").ap()
    aps["xkv"] = nc.dram_tensor("xkv", [C, N], F32, kind="ExternalInput").ap()
    for nm in ("wq", "wk", "wv"):
        aps[nm] = nc.dram_tensor(nm, [C, C], F32, kind="ExternalInput").ap()
    aps["wp"] = nc.dram_tensor("wp", [C, C], F32R, kind="ExternalInput").ap()
    for nm in ("bq", "bk", "bv", "bp"):
        aps[nm] = nc.dram_tensor(nm, [C], F32, kind="ExternalInput").ap()
    aps["out"] = nc.dram_tensor("out", [C, QSH], F32, kind="ExternalOutput").ap()
    with tile.TileContext(nc) as tc:
        _emit(tc, **aps)
    nc.finalize()
    return nc


def kernel(branch1, branch2, Wq, bq, Wk, bk, Wv, bv, Wp, bp, **run_kwargs):
    if "nc" not in _CACHE:
        _CACHE["nc"] = _build()
    nc = _CACHE["nc"]

    x1 = np.ascontiguousarray(np.asarray(branch1, np.float32).reshape(B, C, N))
    x2 = np.ascontiguousarray(np.asarray(branch2, np.float32).reshape(B, C, N))
    consts = {
        "wq": np.ascontiguousarray(Wq, np.float32),
        "wk": np.ascontiguousarray(Wk, np.float32),
        "wv": np.ascontiguousarray(Wv, np.float32),
        "wp": np.ascontiguousarray(Wp, np.float32),
        "bq": np.ascontiguousarray(bq, np.float32),
        "bk": np.ascontiguousarray(bk, np.float32),
        "bv": np.ascontiguousarray(bv, np.float32),
        "bp": np.ascontiguousarray(bp, np.float32),
    }
    in_maps = []
    for core in range(NCORES):
        b, s = divmod(core, NCORES // B)
        in_maps.append({
            "xq": np.ascontiguousarray(x1[b, :, s * QSH:(s + 1) * QSH]),
            "xkv": x2[b],
            **consts,
        })
    res = run_bass_kernel_spmd(nc, in_maps, core_ids=list(range(NCORES)),
                               **run_kwargs)
    out = np.empty((B, C, N), np.float32)
    for core in range(NCORES):
        b, s = divmod(core, NCORES // B)
        out[b, :, s * QSH:(s + 1) * QSH] = res.results[core]["out"]
    if run_kwargs:
        _CACHE["last_result"] = res
    return out.reshape(B, C, D, H, W)


# revision 18
# speedup vs baseline: 1.9292x; 1.0823x over previous
"""Trainium2 Bass kernel for EnhancedCrossAttention3D.

Computes, per batch b:
    q = Wq @ x1 + bq            (x1 = branch1[b] reshaped [C, N])
    k = Wk @ x2 + bk
    v = Wv @ x2 + bv
    attn = softmax((q^T k) / sqrt(C), axis=keys)
    out = Wp @ (attn @ v^T)^T + bp      -> [C, N]

Sharding: 8 cores = 2 batches x 4 query shards of 2048. Each core gets its
full K/V source (branch2[b]) and its query shard; no collectives.

On-core algorithm (flash-style, S^T layout):
    S^T[m, n] = sum_c k[c, m] * qT[c, n]   (m = key index on partitions)
    E = exp(S^T / 8)                       (logits are tiny; no max-sub needed)
    PV[c, n]  = sum_m [v | 1][m, c] * E[m, n]   -> row 64 is the softmax denom
    out[o, n] = Wp @ (PV[0:64] / denom + bv) + bp
(bv is folded in after normalization: attn rows sum to 1.)

Structure: queries are processed in two 1024-wide half-passes so the PV
accumulator needs only 2 PSUM banks, freeing 6 banks for a triple-buffered
S^T pipeline (keeps the PE continuously fed -> HAM stays at full clock).
Matmul operands are bf16 (full-rate streaming + fast weight load); exp runs
on the Scalar engine straight out of PSUM; PSUM accumulation is fp32.
"""

import numpy as np
from contextlib import ExitStack

import concourse.bass as bass
import concourse.mybir as mybir
import concourse.tile as tile
from concourse import bacc
from concourse.bass import ts
from concourse.bass_utils import run_bass_kernel_spmd

B, C, D, H, W = 2, 64, 8, 32, 32
N = D * H * W              # 8192 keys per batch
NCORES = 8
QSH = (B * N) // NCORES    # 2048 queries per core
MCH = N // 128             # 64 key chunks of 128
NH = QSH // 1024           # 2 query half-passes
F32 = mybir.dt.float32
F32R = mybir.dt.float32r
BF16 = mybir.dt.bfloat16
AF = mybir.ActivationFunctionType

_CACHE = {}


def _emit(tc, xq, xkv, wq, wk, wv, wp, bq, bk, bv, bp, out):
    nc = tc.nc
    ctx = ExitStack()
    # bf16/f32r writes below intentionally round fp32; errors wash out in the
    # 8192-term attention sums and sit ~1e-4 of output scale.
    ctx.enter_context(nc.allow_low_precision(reason="bf16 attention operands"))
    const = ctx.enter_context(tc.tile_pool(name="const", bufs=1))
    big = ctx.enter_context(tc.tile_pool(name="big", bufs=1))
    ps3 = ctx.enter_context(tc.tile_pool(name="ps3", bufs=3, space="PSUM"))
    acc_p = ctx.enter_context(tc.tile_pool(name="acc", bufs=1, space="PSUM"))
    ex_pool = ctx.enter_context(tc.tile_pool(name="ex", bufs=4))
    small = ctx.enter_context(tc.tile_pool(name="small", bufs=4))

    # ---- loads (gpsimd DMAs cast f32 -> bf16 in flight) ----
    xq_bf = big.tile([C, QSH], BF16)
    for s in range(2):
        nc.gpsimd.dma_start(out=xq_bf[:, ts(s, QSH // 2)],
                            in_=xq[:, ts(s, QSH // 2)])
    xkv_bf = big.tile([C, N], BF16)
    for s in range(4):
        nc.gpsimd.dma_start(out=xkv_bf[:, ts(s, N // 4)],
                            in_=xkv[:, ts(s, N // 4)])
    wqT = const.tile([C, C], BF16)
    nc.gpsimd.dma_start(out=wqT, in_=wq.rearrange("o c -> c o"))
    wkT = const.tile([C, C], BF16)
    nc.gpsimd.dma_start(out=wkT, in_=wk.rearrange("o c -> c o"))
    wvT = const.tile([C, C], BF16)
    nc.gpsimd.dma_start(out=wvT, in_=wv.rearrange("o c -> c o"))
    wpT = const.tile([C, C], F32R)
    nc.sync.dma_start(out=wpT, in_=wp.rearrange("o c -> c o"))
    bq_sb = const.tile([C, 1], F32)
    nc.sync.dma_start(out=bq_sb, in_=bq.rearrange("(c one) -> c one", one=1))
    bk_sb = const.tile([C, 1], F32)
    nc.sync.dma_start(out=bk_sb, in_=bk.rearrange("(c one) -> c one", one=1))
    bv_sb = const.tile([C, 1], F32)
    nc.sync.dma_start(out=bv_sb, in_=bv.rearrange("(c one) -> c one", one=1))
    bp_sb = const.tile([C, 1], F32)
    nc.sync.dma_start(out=bp_sb, in_=bp.rearrange("(c one) -> c one", one=1))
    ones_f32 = const.tile([128, MCH], F32)
    nc.vector.memset(ones_f32, 1.0)

    # ---- q/k projections (bf16 out via the DVE bias-add) ----
    # Both live twice (partitions 0-63 and 64-127) so the S^T matmuls can be
    # row-packed: two concurrent K=64 matmuls in opposite PE-array halves.
    qT_sb = big.tile([128, QSH], BF16)
    for t in range(QSH // 512):
        pq = ps3.tile([128, 1024], F32, tag="ps")
        nc.tensor.matmul(pq[0:C, 0:512], lhsT=wqT, rhs=xq_bf[:, ts(t, 512)],
                         start=True, stop=True)
        nc.scalar.activation(qT_sb[0:C, ts(t, 512)], pq[0:C, 0:512],
                             AF.Identity, bias=bq_sb, scale=1.0)
        nc.vector.tensor_copy(qT_sb[C:2 * C, ts(t, 512)],
                              qT_sb[0:C, ts(t, 512)])
    k_sb = big.tile([128, N], BF16)
    for t in range(N // 512):
        pk = ps3.tile([128, 1024], F32, tag="ps")
        nc.tensor.matmul(pk[0:C, 0:512], lhsT=wkT, rhs=xkv_bf[:, ts(t, 512)],
                         start=True, stop=True)
        nc.scalar.activation(k_sb[0:C, ts(t, 512)], pk[0:C, 0:512],
                             AF.Identity, bias=bk_sb, scale=1.0)
        nc.vector.tensor_copy(k_sb[C:2 * C, ts(t, 512)], k_sb[0:C, ts(t, 512)])
    # v[m, c] with a ones column (row 64 of PV becomes the softmax denominator)
    v_sb = big.tile([128, MCH, C + 1], BF16)
    nc.vector.tensor_copy(v_sb[:, :, C], ones_f32)

    # ---- flash loop: two query half-passes over all key chunks ----
    for p in range(NH):
        acc = acc_p.tile([C + 1, 1024], F32, tag="acc")
        for i in range(MCH):
            if p == 0 and i % 4 == 0:
                # v chunks i..i+3: 4 matmuls into one psum slot, one cast out
                g = i // 4
                pv = ps3.tile([128, 1024], F32, tag="ps")
                for j in range(4):
                    nc.tensor.matmul(pv[:, ts(j, C)],
                                     lhsT=xkv_bf[:, ts(4 * g + j, 128)],
                                     rhs=wvT, start=True, stop=True)
                nc.vector.tensor_copy(
                    v_sb[:, 4 * g:4 * g + 4, 0:C],
                    pv[:, 0:4 * C].rearrange("p (g c) -> p g c", c=C))
            s_ps = ps3.tile([128, 1024], F32, tag="ps")
            for sub in range(2):
                lo = C * sub
                nc.tensor.matmul(s_ps[:, ts(sub, 512)],
                                 lhsT=k_sb[lo:lo + C, ts(i, 128)],
                                 rhs=qT_sb[lo:lo + C,
                                           p * 1024 + 512 * sub:
                                           p * 1024 + 512 * (sub + 1)],
                                 start=True, stop=True)
            ex = ex_pool.tile([128, 1024], BF16)
            nc.scalar.activation(ex, s_ps, AF.Exp, scale=0.125)
            for sub in range(2):
                nc.tensor.matmul(acc[:, ts(sub, 512)], lhsT=v_sb[:, i, :],
                                 rhs=ex[:, ts(sub, 512)],
                                 start=(i == 0), stop=(i == MCH - 1),
                                 skip_group_check=True)

        # ---- per-pass epilogue: normalize (+bv), project, +bp ----
        denom = small.tile([1, 1024], F32, tag="denom")
        nc.vector.tensor_copy(denom, acc[C:C + 1, :])
        rbd = small.tile([C, 1024], F32, tag="rbd")
        nc.gpsimd.partition_broadcast(rbd, denom)
        rb = small.tile([C, 1024], F32, tag="rb")
        nc.vector.reciprocal(rb, rbd)
        for t in range(2):
            nt = p * 2 + t
            pvn = small.tile([C, 512], F32R, tag="pvn")
            nc.vector.tensor_mul(pvn, acc[0:C, ts(t, 512)], rb[:, ts(t, 512)])
            nc.vector.tensor_scalar_add(pvn, pvn, bv_sb)
            pp = ps3.tile([128, 1024], F32, tag="ps")
            nc.tensor.matmul(pp[0:C, 0:512], lhsT=wpT, rhs=pvn,
                             start=True, stop=True)
            o_sb = small.tile([C, 512], F32, tag="o")
            nc.scalar.activation(o_sb, pp[0:C, 0:512], AF.Identity,
                                 bias=bp_sb, scale=1.0)
            nc.sync.dma_start(out=out[:, ts(nt, 512)], in_=o_sb)
    ctx.close()


def _build():
    nc = bacc.Bacc("TRN2", target_bir_lowering=False, debug=False,
                   num_devices=NCORES)
    aps = {}
    aps["xq"] = nc.dram_tensor("xq", [C, QSH], F32, kind="ExternalInput").ap()
    aps["xkv"] = nc.dram_tensor("xkv", [C, N], F32, kind="ExternalInput").ap()
    for nm in ("wq", "wk", "wv"):
        aps[nm] = nc.dram_tensor(nm, [C, C], F32, kind="ExternalInput").ap()
    aps["wp"] = nc.dram_tensor("wp", [C, C], F32R, kind="ExternalInput").ap()
    for nm in ("bq", "bk", "bv", "bp"):
        aps[nm] = nc.dram_tensor(nm, [C], F32, kind="ExternalInput").ap()
    aps["out"] = nc.dram_tensor("out", [C, QSH], F32, kind="ExternalOutput").ap()
    with tile.TileContext(nc) as tc:
        _emit(tc, **aps)
    nc.finalize()
    return nc


def kernel(branch1, branch2, Wq, bq, Wk, bk, Wv, bv, Wp, bp, **run_kwargs):
    if "nc" not in _CACHE:
        _CACHE["nc"] = _build()
    nc = _CACHE["nc"]

    x1 = np.ascontiguousarray(np.asarray(branch1, np.float32).reshape(B, C, N))
    x2 = np.ascontiguousarray(np.asarray(branch2, np.float32).reshape(B, C, N))
    consts = {
        "wq": np.ascontiguousarray(Wq, np.float32),
        "wk": np.ascontiguousarray(Wk, np.float32),
        "wv": np.ascontiguousarray(Wv, np.float32),
        "wp": np.ascontiguousarray(Wp, np.float32),
        "bq": np.ascontiguousarray(bq, np.float32),
        "bk": np.ascontiguousarray(bk, np.float32),
        "bv": np.ascontiguousarray(bv, np.float32),
        "bp": np.ascontiguousarray(bp, np.float32),
    }
    in_maps = []
    for core in range(NCORES):
        b, s = divmod(core, NCORES // B)
        in_maps.append({
            "xq": np.ascontiguousarray(x1[b, :, s * QSH:(s + 1) * QSH]),
            "xkv": x2[b],
            **consts,
        })
    res = run_bass_kernel_spmd(nc, in_maps, core_ids=list(range(NCORES)),
                               **run_kwargs)
    out = np.empty((B, C, N), np.float32)
    for core in range(NCORES):
        b, s = divmod(core, NCORES // B)
        out[b, :, s * QSH:(s + 1) * QSH] = res.results[core]["out"]
    if run_kwargs:
        _CACHE["last_result"] = res
    return out.reshape(B, C, D, H, W)
